# revision 30
# baseline (speedup 1.0000x reference)
"""Masked multi-head attention on 8 Trainium2 NeuronCores.

Sharding: core c = (b, hg) with b = c // 4, hg = c % 4. Each core computes the
full attention block for batch b restricted to heads [4*hg, 4*hg+4), including
its slice of the QKV projection and of the output projection. The host sums the
4 tensor-parallel partial outputs per batch and adds the output bias.

Shapes are hardcoded for B=2, T=2048, D=1024, H=16, Hd=64, fp32.
"""

import numpy as np
from contextlib import ExitStack

import concourse.bass as bass
import concourse.bacc as bacc
import concourse.mybir as mybir
import concourse.tile as tile
from concourse.bass_utils import run_bass_kernel_spmd

B, T, D = 2, 2048, 1024
H, HD = 16, 64
HL = 4               # heads per core
NCORES = 8
TQ = 512             # query tile (matmul moving free dim)
TK = 128             # key tile
NQT = T // TQ        # 4
NKT = T // TK        # 16
NDT = D // 128       # 8

F32 = mybir.dt.float32
F32R = mybir.dt.float32r
EXP = mybir.ActivationFunctionType.Exp
MULT = mybir.AluOpType.mult

LAST_RESULTS = None  # BassKernelResults of the most recent run (for test.py)


def r(ap):
    return ap if ap.dtype == F32R else ap.bitcast(F32R)


def _build_mha(tc, out_ap, in_aps):
    nc = tc.nc
    x_d = in_aps["x"]          # [T, D]
    wqk_d = in_aps["wqk"]      # [D, 512]  (Qh0|Qh1|Qh2|Qh3|Kh0..Kh3, Q pre-scaled)
    bqk_d = in_aps["bqk"]      # [1, 512]
    wv_d = in_aps["wv"]        # [D, 256]
    bv_d = in_aps["bv"]        # [1, 256]
    wout_d = in_aps["wout"]    # [128, 2, 1024]
    mask_d = in_aps["mask"]    # [128, 896]
    ones_d = in_aps["ones"]    # [128, 512] all-ones

    with ExitStack() as ctx:
        ctx.enter_context(nc.allow_low_precision(reason="fp32r matmul pipeline"))
        const = ctx.enter_context(tc.tile_pool(name="const", bufs=1))
        big = ctx.enter_context(tc.tile_pool(name="big", bufs=1))

        ident = const.tile([128, 128], F32)
        from concourse.masks import make_identity
        make_identity(nc, ident[:])

        # Persistent activations.
        # qkt[:, fb, t]: fb 0,1 = Q^T head pairs (0,1),(2,3); fb 2,3 = K^T pairs.
        # Rows 0:64 = even head of the pair, 64:128 = odd head.
        qkt = big.tile([128, 4, T], F32R)
        # vp[:, kt, h, 0:64] = V[kt*128:+128, h*64:+64]; vp[..., 64] = 1.0
        vp = big.tile([128, NKT, HL, 65], F32R)
        # ot[:, p, t]: normalized attention output^T; rows by head as in qkt
        ot = big.tile([128, 2, T], F32R)

        # ---- Interleaved pipeline over 512-token blocks ----
        # One shared PSUM pool; tags sized so all concurrent users fit in the
        # 8 banks: pt(2) + pq(2) + pv(2) + av0(1) + av1(1).
        with ExitStack() as pctx:
            ps = pctx.enter_context(tc.tile_pool(name="ps", bufs=2, space="PSUM"))
            ps_av = pctx.enter_context(tc.tile_pool(name="ps_av", bufs=1,
                                                    space="PSUM"))
            wpool = pctx.enter_context(tc.tile_pool(name="w", bufs=1))
            xin_p = pctx.enter_context(tc.tile_pool(name="xin", bufs=5))
            xt_p = pctx.enter_context(tc.tile_pool(name="xt", bufs=2))
            pt_p = pctx.enter_context(tc.tile_pool(name="ptile", bufs=6))
            nrm_p = pctx.enter_context(tc.tile_pool(name="nrm", bufs=4))
            ob_p = pctx.enter_context(tc.tile_pool(name="ob", bufs=4))

            # x tiles for the first token block go out first so the PE can
            # start transposing immediately; weights/consts queue behind them
            pre = []
            for ts in range(4):
                xin = xin_p.tile([128, D], F32, tag="xin", name=f"xin_p{ts}")
                nc.sync.dma_start(
                    xin[:], x_d[ts * 128:(ts + 1) * 128, :])
                pre.append(xin)
            wqk = wpool.tile([128, NDT, 512], F32R)
            nc.sync.dma_start(wqk[:], wqk_d.rearrange("(o p) f -> p o f", p=128).bitcast(F32R))
            wv = wpool.tile([128, NDT, 256], F32R)
            nc.sync.dma_start(wv[:], wv_d.rearrange("(o p) f -> p o f", p=128).bitcast(F32R))
            ones = const.tile([1, 512], F32R)
            nc.sync.dma_start(ones[:], ones_d[0:1, :].bitcast(F32R))
            bqk = const.tile([1, 512], F32R)
            nc.sync.dma_start(bqk[:], bqk_d.bitcast(F32R))
            bv = const.tile([1, 256], F32R)
            nc.sync.dma_start(bv[:], bv_d.bitcast(F32R))
            mask = const.tile([128, 896], F32R)
            nc.sync.dma_start(mask[:], mask_d.bitcast(F32R))
            nc.sync.dma_start(
                vp[:, :, :, 64],
                ones_d[:, 0:NKT * HL].rearrange("p (a b) -> p a b", b=HL).bitcast(F32R))
            wout = const.tile([128, 2, 1024], F32R)
            nc.sync.dma_start(wout[:], wout_d.bitcast(F32R))

            def emit_transpose_group(tt, ts, dh, xt):
                """4 PE transposes into one PSUM bank + 1 DVE evacuation."""
                if tt == 0:
                    xin = pre[ts]
                else:
                    xin = xins[(tt, ts)]
                pt = ps.tile([128, 512], F32, tag="pt",
                             name=f"tp_{tt}_{ts}_{dh}")
                for dj in range(4):
                    dt = dh * 4 + dj
                    nc.tensor.transpose(
                        pt[:, dj * 128:(dj + 1) * 128],
                        xin[:, dt * 128:(dt + 1) * 128], ident[:])
                nc.vector.tensor_copy(
                    xt[:, dh * 4:(dh + 1) * 4, ts * 128:(ts + 1) * 128],
                    pt[:].rearrange("p (a b) -> p a b", b=128))

            def emit_x_dma(tt):
                if tt == 0:
                    return
                for ts in range(4):
                    xin = xin_p.tile([128, D], F32, tag="xin",
                                     name=f"xin_{tt}_{ts}")
                    nc.sync.dma_start(
                        xin[:],
                        x_d[tt * TQ + ts * 128 : tt * TQ + (ts + 1) * 128, :])
                    xins[(tt, ts)] = xin

            def transpose_fillers(tt):
                xt = xt_p.tile([128, NDT, TQ], F32R, tag="xt", name=f"xt_{tt}")
                xts[tt] = xt
                return [
                    (lambda tt=tt, ts=ts, dh=dh, xt=xt:
                     emit_transpose_group(tt, ts, dh, xt))
                    for ts in range(4) for dh in range(2)
                ]

            def emit_fb(tt, fb):
                xt = xts[tt]
                pq = ps.tile([128, TQ], F32, tag="pq", name=f"pq_{tt}_{fb}")
                for dt in range(NDT):
                    nc.tensor.matmul(pq[:], r(wqk[:, dt, fb * 128:(fb + 1) * 128]),
                                     r(xt[:, dt, :]),
                                     start=(dt == 0), stop=False)
                nc.tensor.matmul(pq[:], r(bqk[0:1, fb * 128:(fb + 1) * 128]),
                                 r(ones[0:1, :]), start=False, stop=True)
                nc.vector.tensor_copy(qkt[:, fb, tt * TQ:(tt + 1) * TQ], pq[:])

            def emit_v(tt, ts):
                xt = xts[tt]
                pv = ps.tile([128, 512], F32, tag="pv", name=f"pv_{tt}_{ts}")
                for dt in range(NDT):
                    nc.tensor.matmul(pv[:, 0:256],
                                     r(xt[:, dt, ts * 128:(ts + 1) * 128]),
                                     r(wv[:, dt, :]), start=(dt == 0), stop=False)
                nc.tensor.matmul(pv[:, 0:256], r(ones[0:1, 0:128]), r(bv[0:1, :]),
                                 start=False, stop=True)
                nc.vector.tensor_copy(
                    vp[:, tt * 4 + ts, :, 0:64],
                    pv[:, 0:256].rearrange("p (h e) -> p h e", e=HD))

            def emit_b_qkv(tt):
                """Q^T/K^T columns + V rows from the prepared x^T block."""
                for fb in range(4):
                    emit_fb(tt, fb)
                for ts in range(4):
                    emit_v(tt, ts)

            def emit_scores(p, qi, kt):
                """QK^T + exp (+ causal mask on diagonal tiles) -> P^T tiles.

                Diagonal tiles (rr = kt-4qi in 0..3) only need columns
                >= 128*rr; compute cols [c_lo, TQ) with c_lo capped at 256 so
                the fp32r moving dim stays >= 256, and mask-multiply only the
                column range that contains zeros.
                """
                rr = kt - 4 * qi
                c_lo = 0 if rr < 0 else min(128 * rr, 256)
                pts = []
                for a in range(2):          # head within pair
                    rows = slice(64 * a, 64 * a + 64)
                    s = ps.tile([128, TQ], F32, tag=("pq" if a == 0 else "pt"),
                                name=f"s{a}_{p}_{qi}_{kt}")
                    nc.tensor.matmul(
                        s[:, c_lo:], r(qkt[rows, 2 + p, kt * TK:(kt + 1) * TK]),
                        r(qkt[rows, p, qi * TQ + c_lo:(qi + 1) * TQ]),
                        start=True, stop=True)
                    pt = pt_p.tile([128, TQ], F32R, tag=f"pt{a}",
                                   name=f"pt{a}_{p}_{qi}_{kt}")
                    nc.scalar.activation(pt[:, c_lo:], s[:, c_lo:], EXP)
                    if rr >= 0:
                        c0 = (3 - rr) * 128
                        m_lo, m_hi = c_lo, min(128 * rr + 128, TQ)
                        nc.vector.tensor_tensor(
                            pt[:, m_lo:m_hi], pt[:, m_lo:m_hi],
                            mask[:, c0 + m_lo:c0 + m_hi], MULT)
                    pts.append(pt)
                return pts, c_lo

            def emit_c(p, qi, fillers):
                av = [ps_av.tile([128, TQ], F32, tag=f"av{a}",
                                 name=f"av{a}_{p}_{qi}") for a in range(2)]
                nkt = 4 * qi + 4            # causal: k tiles 0 .. 4qi+3
                pts, c_lo = emit_scores(p, qi, 0)
                for kt in range(nkt):
                    # next kt's scores go ahead of this kt's AV, and one unit
                    # of independent PE work (transpose group / out-proj) is
                    # slotted in so the PE isn't gated on the current exp
                    nxt = emit_scores(p, qi, kt + 1) if kt + 1 < nkt else (None, 0)
                    n_pop = min(len(fillers), max(1, -(-len(fillers) // (nkt - kt))))
                    for _ in range(n_pop):
                        fillers.popleft()()
                    for a in range(2):
                        nc.tensor.matmul(
                            av[a][0:65, c_lo:], r(vp[:, kt, 2 * p + a, :]),
                            r(pts[a][:, c_lo:]),
                            start=(kt == 0), stop=(kt == nkt - 1),
                            skip_group_check=True)
                    pts, c_lo = nxt
                # normalize: rows 0:64 are O^T, row 64 is the softmax denom
                for a in range(2):
                    rec = nrm_p.tile([1, TQ], F32R, tag="rec",
                                     name=f"rec_{p}_{qi}_{a}")
                    nc.vector.reciprocal(rec[:], av[a][64:65, :])
                    pb = ps.tile([64, TQ], F32, tag="pt", name=f"pb_{p}_{qi}_{a}")
                    nc.tensor.matmul(pb[:], r(ones[0:1, 0:64]), r(rec[:]),
                                     start=True, stop=True)
                    bc = nrm_p.tile([64, TQ], F32, tag="bc",
                                    name=f"bc_{p}_{qi}_{a}")
                    nc.vector.tensor_copy(bc[:], pb[:])
                    nc.vector.tensor_tensor(
                        ot[64 * a:64 * a + 64, p, qi * TQ:(qi + 1) * TQ],
                        av[a][0:64, :], bc[:], MULT)

            def po_fillers(qi):
                def emit_po(ts, dt):
                    po = ps.tile([128, 512], F32, tag="pv",
                                 name=f"po_{ts}_{dt}")
                    for ft in range(2):
                        nc.tensor.matmul(
                            po[:], r(ot[:, ft, ts * 128:(ts + 1) * 128]),
                            r(wout[:, ft, dt * 512:(dt + 1) * 512]),
                            start=(ft == 0), stop=(ft == 1))
                    ob = ob_p.tile([128, 512], F32, tag="ob",
                                   name=f"ob_{ts}_{dt}")
                    if dt == 0:
                        nc.vector.tensor_copy(ob[:], po[:])
                    else:
                        nc.scalar.copy(ob[:], po[:])
                    nc.sync.dma_start(
                        out_ap[ts * 128:(ts + 1) * 128,
                               dt * 512:(dt + 1) * 512],
                        ob[:])
                return [
                    (lambda ts=ts, dt=dt: emit_po(ts, dt))
                    for ts in range(4 * qi, 4 * qi + 4) for dt in range(2)
                ]

            from collections import deque
            xins, xts = {}, {}
            for f in transpose_fillers(0):  # block 0's x^T up front
                f()
            last = NQT - 1
            for tt in range(NQT):
                if tt < last:
                    emit_b_qkv(tt)
                    fl = deque()
                    emit_x_dma(tt + 1)
                    tps = transpose_fillers(tt + 1)
                    pos = po_fillers(tt - 1) if tt >= 1 else []
                    # alternate so out-proj units land after their normalize
                    # producers have drained, while transposes still finish
                    # before the next block's QKV needs x^T
                    while tps or pos:
                        if tps:
                            fl.append(tps.pop(0))
                        if pos:
                            fl.append(pos.pop(0))
                    emit_c(0, tt, fl)
                    emit_c(1, tt, fl)
                    while fl:
                        fl.popleft()()
                else:
                    # last block: only pair 0's Q/K columns are needed up
                    # front; V, pair 1's columns, and D(tt-1) feed the
                    # C(p0) iteration slots
                    emit_fb(tt, 0)
                    emit_fb(tt, 2)
                    fl = deque()
                    fl.extend([(lambda ts=ts: emit_v(tt, ts))
                               for ts in range(4)])
                    fl.append(lambda: emit_fb(tt, 1))
                    fl.append(lambda: emit_fb(tt, 3))
                    fl.extend(po_fillers(tt - 1))
                    emit_c(0, tt, fl)
                    emit_c(1, tt, fl)
                    while fl:
                        fl.popleft()()
            for f in po_fillers(NQT - 1):
                f()


_CACHE = {}


def _program():
    if "nc" in _CACHE:
        return _CACHE["nc"]
    nc = bacc.Bacc("TRN2", target_bir_lowering=False, debug=False)
    ins = {
        "x": nc.dram_tensor("x", [T, D], F32, kind="ExternalInput").ap(),
        "wqk": nc.dram_tensor("wqk", [D, 512], F32, kind="ExternalInput").ap(),
        "bqk": nc.dram_tensor("bqk", [1, 512], F32, kind="ExternalInput").ap(),
        "wv": nc.dram_tensor("wv", [D, 256], F32, kind="ExternalInput").ap(),
        "bv": nc.dram_tensor("bv", [1, 256], F32, kind="ExternalInput").ap(),
        "wout": nc.dram_tensor("wout", [128, 2, 1024], F32,
                               kind="ExternalInput").ap(),
        "mask": nc.dram_tensor("mask", [128, 896], F32, kind="ExternalInput").ap(),
        "ones": nc.dram_tensor("ones", [128, 512], F32, kind="ExternalInput").ap(),
    }
    out = nc.dram_tensor("out", [T, D], F32, kind="ExternalOutput").ap()
    with tile.TileContext(nc) as tc:
        _build_mha(tc, out, ins)
    nc.compile()
    _CACHE["nc"] = nc
    return nc


def _in_maps(x, Wqkv, bqkv, Wout):
    x = np.asarray(x, dtype=np.float32)
    Wqkv = np.asarray(Wqkv, dtype=np.float32)
    bqkv = np.asarray(bqkv, dtype=np.float32)
    Wout = np.asarray(Wout, dtype=np.float32)
    scale = np.float32(1.0 / np.sqrt(HD))
    mask = (np.arange(128)[:, None] <= np.arange(896)[None, :] - 384).astype(
        np.float32)
    maps = []
    for c in range(NCORES):
        b, hg = c // 4, c % 4
        hs = [4 * hg + i for i in range(HL)]
        q_cols = np.concatenate([Wqkv[:, h * HD:(h + 1) * HD] for h in hs], axis=1)
        k_cols = np.concatenate(
            [Wqkv[:, D + h * HD:D + (h + 1) * HD] for h in hs], axis=1)
        v_cols = np.concatenate(
            [Wqkv[:, 2 * D + h * HD:2 * D + (h + 1) * HD] for h in hs], axis=1)
        bq = np.concatenate([bqkv[h * HD:(h + 1) * HD] for h in hs])
        bk = np.concatenate([bqkv[D + h * HD:D + (h + 1) * HD] for h in hs])
        bv_ = np.concatenate([bqkv[2 * D + h * HD:2 * D + (h + 1) * HD] for h in hs])
        wqk = np.ascontiguousarray(
            np.concatenate([q_cols * scale, k_cols], axis=1))
        bqk = np.concatenate([bq * scale, bk])[None, :]
        wo = np.concatenate([Wout[h * HD:(h + 1) * HD, :] for h in hs], axis=0)
        wo = np.ascontiguousarray(
            wo.reshape(2, 128, D).transpose(1, 0, 2))
        maps.append({
            "x": np.ascontiguousarray(x[b]),
            "wqk": wqk,
            "bqk": np.ascontiguousarray(bqk),
            "wv": np.ascontiguousarray(v_cols),
            "bv": np.ascontiguousarray(bv_[None, :]),
            "wout": wo,
            "mask": mask,
            "ones": np.ones((128, 512), dtype=np.float32),
        })
    return maps


def kernel(x, Wqkv, bqkv, Wout, bout):
    global LAST_RESULTS
    nc = _program()
    maps = _in_maps(x, Wqkv, bqkv, Wout)
    res = run_bass_kernel_spmd(nc, maps, list(range(NCORES)))
    LAST_RESULTS = res
    bout = np.asarray(bout, dtype=np.float32)
    out = np.empty((B, T, D), dtype=np.float32)
    for b in range(B):
        acc = res.results[4 * b]["out"].astype(np.float32)
        for hg in range(1, 4):
            acc = acc + res.results[4 * b + hg]["out"]
        out[b] = acc + bout[None, :]
    return out


# revision 32
# speedup vs baseline: 1.0064x; 1.0064x over previous
"""Masked multi-head attention on 8 Trainium2 NeuronCores.

Sharding: core c = (b, hg) with b = c // 4, hg = c % 4. Each core computes the
full attention block for batch b restricted to heads [4*hg, 4*hg+4), including
its slice of the QKV projection and of the output projection. The host sums the
4 tensor-parallel partial outputs per batch and adds the output bias.

Shapes are hardcoded for B=2, T=2048, D=1024, H=16, Hd=64, fp32.
"""

import numpy as np
from contextlib import ExitStack

import concourse.bass as bass
import concourse.bacc as bacc
import concourse.mybir as mybir
import concourse.tile as tile
from concourse.bass_utils import run_bass_kernel_spmd

B, T, D = 2, 2048, 1024
H, HD = 16, 64
HL = 4               # heads per core
NCORES = 8
TQ = 512             # query tile (matmul moving free dim)
TK = 128             # key tile
NQT = T // TQ        # 4
NKT = T // TK        # 16
NDT = D // 128       # 8

F32 = mybir.dt.float32
F32R = mybir.dt.float32r
EXP = mybir.ActivationFunctionType.Exp
MULT = mybir.AluOpType.mult

LAST_RESULTS = None  # BassKernelResults of the most recent run (for test.py)


def r(ap):
    return ap if ap.dtype == F32R else ap.bitcast(F32R)


def _build_mha(tc, out_ap, in_aps):
    nc = tc.nc
    x_d = in_aps["x"]          # [T, D]
    wqk_d = in_aps["wqk"]      # [D, 512]  (Qh0|Qh1|Qh2|Qh3|Kh0..Kh3, Q pre-scaled)
    bqk_d = in_aps["bqk"]      # [1, 512]
    wv_d = in_aps["wv"]        # [D, 256]
    bv_d = in_aps["bv"]        # [1, 256]
    wout_d = in_aps["wout"]    # [128, 2, 1024]
    mask_d = in_aps["mask"]    # [128, 896]
    ones_d = in_aps["ones"]    # [128, 512] all-ones

    with ExitStack() as ctx:
        ctx.enter_context(nc.allow_low_precision(reason="fp32r matmul pipeline"))
        const = ctx.enter_context(tc.tile_pool(name="const", bufs=1))
        big = ctx.enter_context(tc.tile_pool(name="big", bufs=1))

        ident = const.tile([128, 128], F32)
        from concourse.masks import make_identity
        make_identity(nc, ident[:])

        # Persistent activations.
        # qkt[:, fb, t]: fb 0,1 = Q^T head pairs (0,1),(2,3); fb 2,3 = K^T pairs.
        # Rows 0:64 = even head of the pair, 64:128 = odd head.
        qkt = big.tile([128, 4, T], F32R)
        # vp[:, kt, h, 0:64] = V[kt*128:+128, h*64:+64]; vp[..., 64] = 1.0
        vp = big.tile([128, NKT, HL, 65], F32R)
        # ot[:, p, t]: normalized attention output^T; rows by head as in qkt
        ot = big.tile([128, 2, T], F32R)

        # ---- Interleaved pipeline over 512-token blocks ----
        # One shared PSUM pool; tags sized so all concurrent users fit in the
        # 8 banks: pt(2) + pq(2) + pv(2) + av0(1) + av1(1).
        with ExitStack() as pctx:
            ps = pctx.enter_context(tc.tile_pool(name="ps", bufs=2, space="PSUM"))
            ps_av = pctx.enter_context(tc.tile_pool(name="ps_av", bufs=1,
                                                    space="PSUM"))
            wpool = pctx.enter_context(tc.tile_pool(name="w", bufs=1))
            xin_p = pctx.enter_context(tc.tile_pool(name="xin", bufs=5))
            xt_p = pctx.enter_context(tc.tile_pool(name="xt", bufs=2))
            pt_p = pctx.enter_context(tc.tile_pool(name="ptile", bufs=6))
            nrm_p = pctx.enter_context(tc.tile_pool(name="nrm", bufs=4))
            ob_p = pctx.enter_context(tc.tile_pool(name="ob", bufs=4))

            # x tiles for the first token block go out first so the PE can
            # start transposing immediately; weights/consts queue behind them
            pre = []
            for ts in range(4):
                xin = xin_p.tile([128, D], F32, tag="xin", name=f"xin_p{ts}")
                nc.sync.dma_start(
                    xin[:], x_d[ts * 128:(ts + 1) * 128, :])
                pre.append(xin)
            wqk = wpool.tile([128, NDT, 512], F32R)
            nc.sync.dma_start(wqk[:], wqk_d.rearrange("(o p) f -> p o f", p=128).bitcast(F32R))
            wv = wpool.tile([128, NDT, 256], F32R)
            nc.sync.dma_start(wv[:], wv_d.rearrange("(o p) f -> p o f", p=128).bitcast(F32R))
            ones = const.tile([1, 512], F32R)
            nc.sync.dma_start(ones[:], ones_d[0:1, :].bitcast(F32R))
            bqk = const.tile([1, 512], F32R)
            nc.sync.dma_start(bqk[:], bqk_d.bitcast(F32R))
            bv = const.tile([1, 256], F32R)
            nc.sync.dma_start(bv[:], bv_d.bitcast(F32R))
            mask = const.tile([128, 896], F32R)
            nc.sync.dma_start(mask[:], mask_d.bitcast(F32R))
            nc.sync.dma_start(
                vp[:, :, :, 64],
                ones_d[:, 0:NKT * HL].rearrange("p (a b) -> p a b", b=HL).bitcast(F32R))
            wout = const.tile([128, 2, 1024], F32R)
            nc.sync.dma_start(wout[:], wout_d.bitcast(F32R))

            def emit_transpose_group(tt, ts, dh, xt):
                """4 PE transposes into one PSUM bank + 1 DVE evacuation."""
                if tt == 0:
                    xin = pre[ts]
                else:
                    xin = xins[(tt, ts)]
                pt = ps.tile([128, 512], F32, tag="pt",
                             name=f"tp_{tt}_{ts}_{dh}")
                for dj in range(4):
                    dt = dh * 4 + dj
                    nc.tensor.transpose(
                        pt[:, dj * 128:(dj + 1) * 128],
                        xin[:, dt * 128:(dt + 1) * 128], ident[:])
                nc.vector.tensor_copy(
                    xt[:, dh * 4:(dh + 1) * 4, ts * 128:(ts + 1) * 128],
                    pt[:].rearrange("p (a b) -> p a b", b=128))

            def emit_x_dma(tt):
                if tt == 0:
                    return
                for ts in range(4):
                    xin = xin_p.tile([128, D], F32, tag="xin",
                                     name=f"xin_{tt}_{ts}")
                    nc.sync.dma_start(
                        xin[:],
                        x_d[tt * TQ + ts * 128 : tt * TQ + (ts + 1) * 128, :])
                    xins[(tt, ts)] = xin

            def transpose_fillers(tt):
                xt = xt_p.tile([128, NDT, TQ], F32R, tag="xt", name=f"xt_{tt}")
                xts[tt] = xt
                return [
                    (lambda tt=tt, ts=ts, dh=dh, xt=xt:
                     emit_transpose_group(tt, ts, dh, xt))
                    for ts in range(4) for dh in range(2)
                ]

            def emit_fb(tt, fb):
                xt = xts[tt]
                pq = ps.tile([128, TQ], F32, tag="pq", name=f"pq_{tt}_{fb}")
                for dt in range(NDT):
                    nc.tensor.matmul(pq[:], r(wqk[:, dt, fb * 128:(fb + 1) * 128]),
                                     r(xt[:, dt, :]),
                                     start=(dt == 0), stop=False)
                nc.tensor.matmul(pq[:], r(bqk[0:1, fb * 128:(fb + 1) * 128]),
                                 r(ones[0:1, :]), start=False, stop=True)
                nc.vector.tensor_copy(qkt[:, fb, tt * TQ:(tt + 1) * TQ], pq[:])

            def emit_v(tt, ts):
                xt = xts[tt]
                pv = ps.tile([128, 512], F32, tag="pv", name=f"pv_{tt}_{ts}")
                for dt in range(NDT):
                    nc.tensor.matmul(pv[:, 0:256],
                                     r(xt[:, dt, ts * 128:(ts + 1) * 128]),
                                     r(wv[:, dt, :]), start=(dt == 0), stop=False)
                nc.tensor.matmul(pv[:, 0:256], r(ones[0:1, 0:128]), r(bv[0:1, :]),
                                 start=False, stop=True)
                nc.vector.tensor_copy(
                    vp[:, tt * 4 + ts, :, 0:64],
                    pv[:, 0:256].rearrange("p (h e) -> p h e", e=HD))

            def emit_b_qkv(tt):
                """Q^T/K^T columns + V rows from the prepared x^T block."""
                for fb in range(4):
                    emit_fb(tt, fb)
                for ts in range(4):
                    emit_v(tt, ts)

            def emit_scores(p, qi, kt):
                """QK^T + exp (+ causal mask on diagonal tiles) -> P^T tiles.

                Diagonal tiles (rr = kt-4qi in 0..3) only need columns
                >= 128*rr; compute cols [c_lo, TQ) with c_lo capped at 256 so
                the fp32r moving dim stays >= 256, and mask-multiply only the
                column range that contains zeros.
                """
                rr = kt - 4 * qi
                c_lo = 0 if rr < 0 else min(128 * rr, 256)
                pts = []
                for a in range(2):          # head within pair
                    rows = slice(64 * a, 64 * a + 64)
                    s = ps.tile([128, TQ], F32, tag=("pq" if a == 0 else "pt"),
                                name=f"s{a}_{p}_{qi}_{kt}")
                    nc.tensor.matmul(
                        s[:, c_lo:], r(qkt[rows, 2 + p, kt * TK:(kt + 1) * TK]),
                        r(qkt[rows, p, qi * TQ + c_lo:(qi + 1) * TQ]),
                        start=True, stop=True)
                    pt = pt_p.tile([128, TQ], F32R, tag=f"pt{a}",
                                   name=f"pt{a}_{p}_{qi}_{kt}")
                    nc.scalar.activation(pt[:, c_lo:], s[:, c_lo:], EXP)
                    if rr >= 0:
                        c0 = (3 - rr) * 128
                        m_lo, m_hi = c_lo, min(128 * rr + 128, TQ)
                        nc.vector.tensor_tensor(
                            pt[:, m_lo:m_hi], pt[:, m_lo:m_hi],
                            mask[:, c0 + m_lo:c0 + m_hi], MULT)
                    pts.append(pt)
                return pts, c_lo

            def emit_c(p, qi, fillers):
                av = [ps_av.tile([128, TQ], F32, tag=f"av{a}",
                                 name=f"av{a}_{p}_{qi}") for a in range(2)]
                nkt = 4 * qi + 4            # causal: k tiles 0 .. 4qi+3
                pts, c_lo = emit_scores(p, qi, 0)
                for kt in range(nkt):
                    # next kt's scores go ahead of this kt's AV, and one unit
                    # of independent PE work (transpose group / out-proj) is
                    # slotted in so the PE isn't gated on the current exp
                    nxt = emit_scores(p, qi, kt + 1) if kt + 1 < nkt else (None, 0)
                    n_pop = min(len(fillers), max(1, -(-len(fillers) // (nkt - kt))))
                    for _ in range(n_pop):
                        fillers.popleft()()
                    for a in range(2):
                        nc.tensor.matmul(
                            av[a][0:65, c_lo:], r(vp[:, kt, 2 * p + a, :]),
                            r(pts[a][:, c_lo:]),
                            start=(kt == 0), stop=(kt == nkt - 1),
                            skip_group_check=True)
                    pts, c_lo = nxt
                # normalize: rows 0:64 are O^T, row 64 is the softmax denom
                for a in range(2):
                    rec = nrm_p.tile([1, TQ], F32R, tag="rec",
                                     name=f"rec_{p}_{qi}_{a}")
                    nc.vector.reciprocal(rec[:], av[a][64:65, :])
                    pb = ps.tile([64, TQ], F32, tag="pt", name=f"pb_{p}_{qi}_{a}")
                    nc.tensor.matmul(pb[:], r(ones[0:1, 0:64]), r(rec[:]),
                                     start=True, stop=True)
                    bc = nrm_p.tile([64, TQ], F32, tag="bc",
                                    name=f"bc_{p}_{qi}_{a}")
                    nc.vector.tensor_copy(bc[:], pb[:])
                    nc.vector.tensor_tensor(
                        ot[64 * a:64 * a + 64, p, qi * TQ:(qi + 1) * TQ],
                        av[a][0:64, :], bc[:], MULT)

            def po_fillers(qi):
                def emit_po(ts, dt):
                    po = ps.tile([128, 512], F32, tag="pv",
                                 name=f"po_{ts}_{dt}")
                    for ft in range(2):
                        nc.tensor.matmul(
                            po[:], r(ot[:, ft, ts * 128:(ts + 1) * 128]),
                            r(wout[:, ft, dt * 512:(dt + 1) * 512]),
                            start=(ft == 0), stop=(ft == 1))
                    ob = ob_p.tile([128, 512], F32, tag="ob",
                                   name=f"ob_{ts}_{dt}")
                    nc.vector.tensor_copy(ob[:], po[:])
                    nc.sync.dma_start(
                        out_ap[ts * 128:(ts + 1) * 128,
                               dt * 512:(dt + 1) * 512],
                        ob[:])
                return [
                    (lambda ts=ts, dt=dt: emit_po(ts, dt))
                    for ts in range(4 * qi, 4 * qi + 4) for dt in range(2)
                ]

            from collections import deque
            xins, xts = {}, {}
            for f in transpose_fillers(0):  # block 0's x^T up front
                f()
            last = NQT - 1
            for tt in range(NQT):
                if tt < last:
                    emit_b_qkv(tt)
                    fl = deque()
                    emit_x_dma(tt + 1)
                    tps = transpose_fillers(tt + 1)
                    pos = po_fillers(tt - 1) if tt >= 1 else []
                    # alternate so out-proj units land after their normalize
                    # producers have drained, while transposes still finish
                    # before the next block's QKV needs x^T
                    while tps or pos:
                        if tps:
                            fl.append(tps.pop(0))
                        if pos:
                            fl.append(pos.pop(0))
                    emit_c(0, tt, fl)
                    emit_c(1, tt, fl)
                    while fl:
                        fl.popleft()()
                else:
                    # last block: only pair 0's Q/K columns are needed up
                    # front; V, pair 1's columns, and D(tt-1) feed the
                    # C(p0) iteration slots
                    emit_fb(tt, 0)
                    emit_fb(tt, 2)
                    fl = deque()
                    fl.extend([(lambda ts=ts: emit_v(tt, ts))
                               for ts in range(4)])
                    fl.append(lambda: emit_fb(tt, 1))
                    fl.append(lambda: emit_fb(tt, 3))
                    fl.extend(po_fillers(tt - 1))
                    emit_c(0, tt, fl)
                    emit_c(1, tt, fl)
                    while fl:
                        fl.popleft()()
            for f in po_fillers(NQT - 1):
                f()


_CACHE = {}


def _program():
    if "nc" in _CACHE:
        return _CACHE["nc"]
    nc = bacc.Bacc("TRN2", target_bir_lowering=False, debug=False)
    ins = {
        "x": nc.dram_tensor("x", [T, D], F32, kind="ExternalInput").ap(),
        "wqk": nc.dram_tensor("wqk", [D, 512], F32, kind="ExternalInput").ap(),
        "bqk": nc.dram_tensor("bqk", [1, 512], F32, kind="ExternalInput").ap(),
        "wv": nc.dram_tensor("wv", [D, 256], F32, kind="ExternalInput").ap(),
        "bv": nc.dram_tensor("bv", [1, 256], F32, kind="ExternalInput").ap(),
        "wout": nc.dram_tensor("wout", [128, 2, 1024], F32,
                               kind="ExternalInput").ap(),
        "mask": nc.dram_tensor("mask", [128, 896], F32, kind="ExternalInput").ap(),
        "ones": nc.dram_tensor("ones", [128, 512], F32, kind="ExternalInput").ap(),
    }
    out = nc.dram_tensor("out", [T, D], F32, kind="ExternalOutput").ap()
    with tile.TileContext(nc) as tc:
        _build_mha(tc, out, ins)
    nc.compile()
    _CACHE["nc"] = nc
    return nc


def _in_maps(x, Wqkv, bqkv, Wout):
    x = np.asarray(x, dtype=np.float32)
    Wqkv = np.asarray(Wqkv, dtype=np.float32)
    bqkv = np.asarray(bqkv, dtype=np.float32)
    Wout = np.asarray(Wout, dtype=np.float32)
    scale = np.float32(1.0 / np.sqrt(HD))
    mask = (np.arange(128)[:, None] <= np.arange(896)[None, :] - 384).astype(
        np.float32)
    maps = []
    for c in range(NCORES):
        b, hg = c // 4, c % 4
        hs = [4 * hg + i for i in range(HL)]
        q_cols = np.concatenate([Wqkv[:, h * HD:(h + 1) * HD] for h in hs], axis=1)
        k_cols = np.concatenate(
            [Wqkv[:, D + h * HD:D + (h + 1) * HD] for h in hs], axis=1)
        v_cols = np.concatenate(
            [Wqkv[:, 2 * D + h * HD:2 * D + (h + 1) * HD] for h in hs], axis=1)
        bq = np.concatenate([bqkv[h * HD:(h + 1) * HD] for h in hs])
        bk = np.concatenate([bqkv[D + h * HD:D + (h + 1) * HD] for h in hs])
        bv_ = np.concatenate([bqkv[2 * D + h * HD:2 * D + (h + 1) * HD] for h in hs])
        wqk = np.ascontiguousarray(
            np.concatenate([q_cols * scale, k_cols], axis=1))
        bqk = np.concatenate([bq * scale, bk])[None, :]
        wo = np.concatenate([Wout[h * HD:(h + 1) * HD, :] for h in hs], axis=0)
        wo = np.ascontiguousarray(
            wo.reshape(2, 128, D).transpose(1, 0, 2))
        maps.append({
            "x": np.ascontiguousarray(x[b]),
            "wqk": wqk,
            "bqk": np.ascontiguousarray(bqk),
            "wv": np.ascontiguousarray(v_cols),
            "bv": np.ascontiguousarray(bv_[None, :]),
            "wout": wo,
            "mask": mask,
            "ones": np.ones((128, 512), dtype=np.float32),
        })
    return maps


def kernel(x, Wqkv, bqkv, Wout, bout):
    global LAST_RESULTS
    nc = _program()
    maps = _in_maps(x, Wqkv, bqkv, Wout)
    res = run_bass_kernel_spmd(nc, maps, list(range(NCORES)))
    LAST_RESULTS = res
    bout = np.asarray(bout, dtype=np.float32)
    out = np.empty((B, T, D), dtype=np.float32)
    for b in range(B):
        acc = res.results[4 * b]["out"].astype(np.float32)
        for hg in range(1, 4):
            acc = acc + res.results[4 * b + hg]["out"]
        out[b] = acc + bout[None, :]
    return out
